# revision 1
# baseline (speedup 1.0000x reference)
"""Multi-head attention (16 heads, d_model=1024, T=2048, B=2) on 8 trn2 NeuronCores.

Sharding: core c -> batch c//4, head-group c%4 (4 heads of 64 dims each).
Each core computes q/k/v projections for its 4 heads on its batch, full
softmax attention for those heads, and a partial output projection
(row-parallel Wo).  Host sums the 4 partials per batch and adds the bias.

All matmuls run in bf16 (fp32 PSUM accumulation).  Scores are computed
transposed (ST[u,t] = sum_s k[u,s] q[t,s]) so that:
  - softmax sum over u is obtained from the attention*V matmul itself by
    appending a ones-column to V (row 64 of the av output = denominator),
  - no transposes of the 2048x2048 attention matrix are ever needed.
The 1/sqrt(d_model) scale is folded into Wq on the host.
"""

import math
import types
import sys

import numpy as np
import ml_dtypes

B = 2
T = 1024 * 2  # 2048 sequence
K = 1024  # model dim
H = 16  # heads
S = K // H  # 64 head dim
HPC = 4  # heads per core
NCORES = 8

_BF16 = ml_dtypes.bfloat16
import os as _os

_RECIP_FAST = _os.environ.get("RECIP_FAST", "0") == "1"


def _install_drain_split_patch():
    """walrus in this container rejects >1 sync-wait on the final tile drain;
    split the waits one-per-drain-instruction (all before the end barrier)."""
    import concourse.tile as tile
    import concourse.mybir as mybir
    from concourse.vector_clock import ScopedClock

    if getattr(tile.TileContext, "_drain_split_patched", False):
        return

    def _patched_dab(self, tick_clock, wait_clock):
        drain_inst = self.nc.sync.drain()
        wait_clock.add_sem_waits(
            drain_inst.ins, ScopedClock({None: tick_clock.global_clock})
        )
        si = drain_inst.ins.sync_info
        waits = list(si.on_wait) if si is not None else []
        if len(waits) > 1:
            si.on_wait = waits[:1]
            for w in waits[1:]:
                extra = self.nc.sync.drain()
                esi = extra.ins.sync_info
                if esi is None:
                    extra.ins.sync_info = mybir.SyncInfo(on_update=[], on_wait=[w])
                else:
                    esi.on_wait = [w]
        self.nc.all_engine_barrier()
        assert self.sems is not None
        popped = self.nc._tile_sem_poison_stack.pop()
        assert popped is self._sem_poison
        self.nc.clear_and_free_semaphores(list(self.sems.allocated().values()))
        self.nc.all_engine_barrier()

    tile.TileContext._drain_and_barrier = _patched_dab
    tile.TileContext._drain_split_patched = True


def build_program():
    """Build the single-core Bass program (same program on all 8 cores)."""
    import concourse.bass as bass
    import concourse.mybir as mybir
    import concourse.tile as tile
    from concourse import bacc

    dt = mybir.dt
    AF = mybir.ActivationFunctionType
    Alu = mybir.AluOpType

    nc = bacc.Bacc()

    xT = nc.dram_tensor("xT", [K, T], dt.bfloat16, kind="ExternalInput")
    wq = nc.dram_tensor("wq", [K, 256], dt.bfloat16, kind="ExternalInput")
    wk = nc.dram_tensor("wk", [K, 256], dt.bfloat16, kind="ExternalInput")
    wv = nc.dram_tensor("wv", [K, 256], dt.bfloat16, kind="ExternalInput")
    wo = nc.dram_tensor("wo", [256, K], dt.bfloat16, kind="ExternalInput")
    ident = nc.dram_tensor("ident", [128, 128], dt.bfloat16, kind="ExternalInput")
    out = nc.dram_tensor("out", [T, K], dt.float32, kind="ExternalOutput")

    KT = K // 128  # 8 k tiles
    TB = T // 128  # 16 t blocks
    VW = 65  # v columns per head incl ones col
    VROW = HPC * VW  # 260 per u-block row

    with tile.TileContext(nc) as tc:
        with (
            tc.tile_pool(name="xt", bufs=KT) as xt_pool,
            tc.tile_pool(name="w", bufs=3) as w_pool,
            tc.tile_pool(name="wo", bufs=2) as wo_pool,
            tc.tile_pool(name="qk", bufs=4) as qk_pool,
            tc.tile_pool(name="v", bufs=1) as v_pool,
            tc.tile_pool(name="yt", bufs=2) as yt_pool,
            tc.tile_pool(name="e", bufs=4) as e_pool,
            tc.tile_pool(name="dinv", bufs=4) as dinv_pool,
            tc.tile_pool(name="avs", bufs=4) as avs_pool,
            tc.tile_pool(name="osb", bufs=3) as osb_pool,
            tc.tile_pool(name="ps1", bufs=4, space="PSUM") as ps1_pool,
            tc.tile_pool(name="pst", bufs=2, space="PSUM") as pst_pool,
        ):
            # ---- loads ----
            xt = []
            for a in range(KT):
                t = xt_pool.tile([128, T], dt.bfloat16, tag="xt")
                nc.sync.dma_start(t[:], xT[a * 128 : (a + 1) * 128, :])
                xt.append(t)

            w_sb = {}
            for name, dram in (("q", wq), ("k", wk), ("v", wv)):
                t = w_pool.tile([128, KT * 256], dt.bfloat16, tag="w")
                nc.sync.dma_start(
                    t[:].rearrange("p (a c) -> p a c", a=KT),
                    dram.rearrange("(a p) c -> p a c", p=128),
                )
                w_sb[name] = t

            wo_sb = []
            for i in range(2):
                t = wo_pool.tile([128, K], dt.bfloat16, tag="wo")
                nc.sync.dma_start(t[:], wo[i * 128 : (i + 1) * 128, :])
                wo_sb.append(t)

            # v with ones columns: [128, 16 u-blocks * (4 heads * 65)]
            v_sb = v_pool.tile([128, TB * VROW], dt.bfloat16, tag="v")
            ones_ap = v_sb[:].rearrange(
                "p (u h c) -> p u h c", u=TB, h=HPC
            )[:, :, :, S : S + 1]
            nc.vector.memset(ones_ap, 1.0)

            # selector matrices for PE-based partition broadcast of 1/D:
            # dinv rows live at partitions j = hl*2 + c; SEL_c.T @ dinv puts
            # row (hl*2+c) broadcast over out partitions hl*64..hl*64+63.
            sel = []
            for c in range(2):
                s = v_pool.tile([128, 128], dt.bfloat16, tag=f"sel{c}", name=f"sel_{c}")
                nc.vector.memset(s[:], 0.0)
                nc.vector.memset(s[32 * c : 32 * c + 1, 0:64], 1.0)
                nc.vector.memset(s[64 + 32 * c : 64 + 32 * c + 1, 64:128], 1.0)
                sel.append(s)

            identity = v_pool.tile([128, 128], dt.bfloat16, tag="ident")
            nc.sync.dma_start(identity[:], ident[:])

            # ---- projections ----
            # v via vT (weights stationary like q/k) then PE transposes into
            # the [u, s|1] layout the attention*V matmuls need.
            def v_proj():
                for sh in range(2):  # s halves of 128 (2 heads each)
                    vt = w_pool.tile(
                        [128, T], dt.bfloat16, tag=f"vt{sh}", name=f"vt_{sh}"
                    )
                    pss = [
                        ps1_pool.tile(
                            [128, 512], dt.float32, tag="ps1", name=f"pv_{sh}_{c}"
                        )
                        for c in range(4)
                    ]
                    for a in range(KT):
                        for c in range(4):
                            nc.tensor.matmul(
                                pss[c][:],
                                w_sb["v"][:, a * 256 + sh * 128 : a * 256 + sh * 128 + 128],
                                xt[a][:, c * 512 : (c + 1) * 512],
                                start=(a == 0),
                                stop=(a == KT - 1),
                            )
                    for c in range(4):
                        tsl = slice(c * 512, (c + 1) * 512)
                        if c % 2 == 0:
                            nc.vector.tensor_copy(vt[:, tsl], pss[c][:])
                        else:
                            nc.scalar.copy(vt[:, tsl], pss[c][:])
                    for ub in range(TB):
                        tp = pst_pool.tile(
                            [128, 128], dt.bfloat16, tag="st", name=f"vtp_{sh}_{ub}"
                        )
                        nc.tensor.transpose(
                            tp[:], vt[:, ub * 128 : (ub + 1) * 128], identity[:]
                        )
                        dst = v_sb[
                            :, ub * VROW + 2 * sh * VW : ub * VROW + (2 * sh + 2) * VW
                        ].rearrange("p (h c) -> p h c", h=2)[:, :, 0:S]
                        nc.vector.tensor_copy(
                            dst, tp[:].rearrange("p (h c) -> p h c", c=S)
                        )

            qt_sb = [None, None]  # per head pair: [128, T], rows 2x64 head dims
            kt_sb = [None, None]

            def qk_proj(hp):
                qt = qk_pool.tile([128, T], dt.bfloat16, tag="qk", name=f"qt_{hp}")
                kt = qk_pool.tile([128, T], dt.bfloat16, tag="qk", name=f"kt_{hp}")
                qt_sb[hp] = qt
                kt_sb[hp] = kt
                for which, dst in (("q", qt), ("k", kt)):
                    # weights stay stationary across the 4 output chunks
                    pss = [
                        ps1_pool.tile(
                            [128, 512], dt.float32, tag="ps1", name=f"pp_{hp}_{which}_{c}"
                        )
                        for c in range(4)
                    ]
                    for a in range(KT):
                        for c in range(4):
                            nc.tensor.matmul(
                                pss[c][:],
                                w_sb[which][:, a * 256 + hp * 128 : a * 256 + hp * 128 + 128],
                                xt[a][:, c * 512 : (c + 1) * 512],
                                start=(a == 0),
                                stop=(a == KT - 1),
                            )
                    for c in range(4):
                        tsl = slice(c * 512, (c + 1) * 512)
                        if which == "q":
                            nc.scalar.copy(dst[:, tsl], pss[c][:])
                        else:
                            nc.vector.tensor_copy(dst[:, tsl], pss[c][:])

            # ---- attention (t-half outer; heads of a pair interleaved) ----
            # yt[hp]: [128, T] bf16, rows (h%2)*64+s hold y^T for the pair
            yt_sb = [
                yt_pool.tile([128, T], dt.bfloat16, tag="yt", name=f"yt_{hp}")
                for hp in range(2)
            ]

            def attention_uloop(hp, th):
                t0 = th * 1024
                av = {}
                for hl in range(2):
                    for c in range(2):
                        av[(hl, c)] = ps1_pool.tile(
                            [65, 512], dt.float32, tag="ps1", name=f"av_{hp}_{th}_{hl}_{c}"
                        )
                for ub in range(TB):
                    es = []
                    for hl in range(2):
                        st = pst_pool.tile(
                            [128, 1024], dt.float32, tag="st", name=f"st_{hp}_{th}_{ub}_{hl}"
                        )
                        for c in range(2):
                            nc.tensor.matmul(
                                st[:, c * 512 : (c + 1) * 512],
                                kt_sb[hp][hl * 64 : (hl + 1) * 64, ub * 128 : (ub + 1) * 128],
                                qt_sb[hp][hl * 64 : (hl + 1) * 64, t0 + c * 512 : t0 + (c + 1) * 512],
                                start=True,
                                stop=True,
                            )
                        e = e_pool.tile(
                            [128, 1024], dt.bfloat16, tag="e", name=f"e_{hp}_{th}_{ub}_{hl}"
                        )
                        nc.scalar.activation(e[:], st[:], AF.Exp)
                        es.append(e)
                    for hl in range(2):
                        lh = 2 * hp + hl  # local head index 0..3
                        voff = ub * VROW + lh * VW
                        for c in range(2):
                            nc.tensor.matmul(
                                av[(hl, c)][:],
                                v_sb[:, voff : voff + VW],
                                es[hl][:, c * 512 : (c + 1) * 512],
                                start=(ub == 0),
                                stop=(ub == TB - 1),
                            )
                # evacuate av psum; pack the four D rows (hl,c) onto 32-aligned
                # partitions of one tile so a single multi-lane reciprocal
                # covers them all, off the PE critical path.
                avss = {}
                for hl in range(2):
                    avs = avs_pool.tile(
                        [64, 1024], dt.float32, tag="avs", name=f"avs_{hp}_{th}_{hl}"
                    )
                    for c in range(2):
                        csl = slice(c * 512, (c + 1) * 512)
                        if c == 0:
                            nc.vector.tensor_copy(avs[:, csl], av[(hl, c)][0:64, :])
                        else:
                            nc.scalar.copy(avs[:, csl], av[(hl, c)][0:64, :])
                    avss[hl] = avs
                # D rows parked at partitions {0,32,64,96}; unused partitions
                # preset to 1.0 so the full-tile reciprocal stays finite.
                drows = dinv_pool.tile(
                    [128, 512], dt.float32, tag="drows", name=f"drows_{hp}_{th}"
                )
                nc.gpsimd.memset(drows[:], 1.0)
                for hl in range(2):
                    for c in range(2):
                        r = hl * 64 + c * 32
                        if c == 0:
                            nc.vector.tensor_copy(
                                drows[r : r + 1, :], av[(hl, c)][64:65, :]
                            )
                        else:
                            nc.scalar.copy(drows[r : r + 1, :], av[(hl, c)][64:65, :])
                dinv = dinv_pool.tile(
                    [128, 512], dt.float32, tag="dinvf", name=f"dinvf_{hp}_{th}"
                )
                nc.vector.reciprocal(dinv[:], drows[:])
                dinvb = dinv_pool.tile(
                    [128, 512], dt.bfloat16, tag="dinvb", name=f"dinvb_{hp}_{th}"
                )
                nc.vector.tensor_copy(dinvb[:], dinv[:])
                return avss, dinvb

            def normalize_flush(hp, th, pend):
                t0 = th * 1024
                yt = yt_sb[hp]
                avss, dinvb = pend
                for c in range(2):
                    # broadcast 1/D of both heads over 128 partitions at once
                    dbp = ps1_pool.tile(
                        [128, 512], dt.float32, tag="ps1", name=f"dbp_{hp}_{th}_{c}"
                    )
                    nc.tensor.matmul(
                        dbp[:], sel[c][:], dinvb[:], start=True, stop=True
                    )
                    for hl in range(2):
                        nc.vector.tensor_tensor(
                            yt[hl * 64 : (hl + 1) * 64, t0 + c * 512 : t0 + (c + 1) * 512],
                            avss[hl][0:64, c * 512 : (c + 1) * 512],
                            dbp[hl * 64 : (hl + 1) * 64, :],
                            op=Alu.mult,
                        )

            def outproj(tb):
                osb = osb_pool.tile([128, K], dt.float32, tag="osb", name=f"osb_{tb}")
                pso = [
                    ps1_pool.tile([128, 512], dt.float32, tag="ps1", name=f"pso_{tb}_{i}")
                    for i in range(2)
                ]
                for hp in range(2):
                    for oc in range(2):
                        nc.tensor.matmul(
                            pso[oc][:],
                            yt_sb[hp][:, tb * 128 : (tb + 1) * 128],
                            wo_sb[hp][:, oc * 512 : (oc + 1) * 512],
                            start=(hp == 0),
                            stop=(hp == 1),
                        )
                for oc in range(2):
                    if tb % 2 == 0:
                        nc.vector.tensor_copy(osb[:, oc * 512 : (oc + 1) * 512], pso[oc][:])
                    else:
                        nc.scalar.copy(osb[:, oc * 512 : (oc + 1) * 512], pso[oc][:])
                nc.sync.dma_start(out[tb * 128 : (tb + 1) * 128, :], osb[:])

            qk_proj(0)
            v_proj()
            pend00 = attention_uloop(0, 0)
            qk_proj(1)
            pend10 = attention_uloop(1, 0)
            normalize_flush(0, 0, pend00)
            normalize_flush(1, 0, pend10)
            for tb in range(0, 8):
                outproj(tb)
            pend01 = attention_uloop(0, 1)
            pend11 = attention_uloop(1, 1)
            normalize_flush(0, 1, pend01)
            normalize_flush(1, 1, pend11)
            for tb in range(8, 16):
                outproj(tb)

    nc.finalize()
    return nc


def _prepare_in_maps(x, Wq, Wk, Wv, Wo):
    scale = 1.0 / math.sqrt(K)
    xT = [np.ascontiguousarray(x[b].T).astype(_BF16) for b in range(B)]
    in_maps = []
    for c in range(NCORES):
        b = c // 4
        g = c % 4
        sl = slice(g * 256, (g + 1) * 256)
        in_maps.append(
            {
                "ident": np.eye(128, dtype=_BF16),
                "xT": xT[b],
                "wq": np.ascontiguousarray((Wq[sl, :].astype(np.float64) * scale).T).astype(_BF16),
                "wk": np.ascontiguousarray(Wk[sl, :].T).astype(_BF16),
                "wv": np.ascontiguousarray(Wv[sl, :].T).astype(_BF16),
                "wo": np.ascontiguousarray(Wo[:, sl].T).astype(_BF16),
            }
        )
    return in_maps


def _gather(results, bo):
    out = np.zeros((B, T, K), dtype=np.float32)
    for b in range(B):
        acc = np.zeros((T, K), dtype=np.float32)
        for g in range(4):
            acc += results[b * 4 + g]["out"].astype(np.float32)
        out[b] = acc + bo.astype(np.float32)[None, :]
    return out


def _maybe_enable_ldw_opt():
    import os
    import concourse.bass_utils as bu

    if os.environ.get("LDWOPT", "0") != "1":
        return
    if getattr(bu, "_ldwopt_patched", False):
        return
    orig = bu.run_command

    def patched(argv, **kw):
        argv = [
            "--enable-ldw-opt=true" if a == "--enable-ldw-opt=false" else a
            for a in argv
        ]
        return orig(argv, **kw)

    bu.run_command = patched
    bu._ldwopt_patched = True


def run(x, Wq, Wk, Wv, Wo, bo, trace=False, tmpdir=None):
    from concourse.bass_utils import run_bass_kernel_spmd

    _maybe_enable_ldw_opt()

    nc = build_program()
    in_maps = _prepare_in_maps(
        np.asarray(x), np.asarray(Wq), np.asarray(Wk), np.asarray(Wv), np.asarray(Wo)
    )
    res = run_bass_kernel_spmd(
        nc, in_maps, list(range(NCORES)), trace=trace, tmpdir=tmpdir
    )
    out = _gather(res.results, np.asarray(bo))
    return out, res


def kernel(x, Wq, Wk, Wv, Wo, bo):
    out, _ = run(x, Wq, Wk, Wv, Wo, bo, trace=False)
    return out



# revision 3
# speedup vs baseline: 1.3644x; 1.3644x over previous
"""Multi-head attention (16 heads, d_model=1024, T=2048, B=2) on 8 trn2 NeuronCores.

Sharding: core c -> batch c//4, head-group c%4 (4 heads of 64 dims each).
Each core computes q/k/v projections for its 4 heads on its batch, full
softmax attention for those heads, and a partial output projection
(row-parallel Wo).  Host sums the 4 partials per batch and adds the bias.

v2 design (vs baseline):
  - score matmuls (contract=64) emitted as adjacent h0/h64 row-tile pairs
    -> run concurrently on the two 64x128 PE tiles (~2x).
  - attn*V matmuls col-tiled: h0 -> psum partitions 0-63, h1 -> 64-127 of
    the same bank (~2x, and AV psum shrinks to 2 banks).
  - softmax denominators: fp16 running accumulation of the exp tiles on
    DVE/GpSimd, then a single ones-vector matmul per (head, chunk) reduces
    the 128 partitions; 1/D via ACT Ln + Exp(-x) (same act table set as
    the softmax Exp, so no table reloads).
  - V projection computed directly in [u, s] orientation (stationary xT
    block, moving Wv) - no PE transposes.
  - static software pipeline: projection / output chunks are interleaved
    into the ACT(exp)-bound attention phases as fillers; each phase's
    normalization is emitted as the first filler of the next phase so the
    PE never waits on the 1/D chain.
PSUM budget: st 2x[128,1024] (4 banks) + av 2x[128,512] (2) + pj 2 (2).
"""

import math
import os as _os

import numpy as np
import ml_dtypes

B = 2
T = 2048
K = 1024
H = 16
S = K // H  # 64
NCORES = 8
TB = T // 128  # 16 u-blocks
KT = K // 128  # 8 contraction tiles

_BF16 = ml_dtypes.bfloat16
_GPS_ACC = _os.environ.get("GPS_ACC", "1") == "1"


def _install_drain_split_patch():
    """walrus in this container rejects >1 sync-wait on the final tile drain;
    split the waits one-per-drain-instruction (all before the end barrier)."""
    import concourse.tile as tile
    import concourse.mybir as mybir
    from concourse.vector_clock import ScopedClock

    if getattr(tile.TileContext, "_drain_split_patched", False):
        return

    def _patched_dab(self, tick_clock, wait_clock):
        drain_inst = self.nc.sync.drain()
        wait_clock.add_sem_waits(
            drain_inst.ins, ScopedClock({None: tick_clock.global_clock})
        )
        si = drain_inst.ins.sync_info
        waits = list(si.on_wait) if si is not None else []
        if len(waits) > 1:
            si.on_wait = waits[:1]
            for w in waits[1:]:
                extra = self.nc.sync.drain()
                esi = extra.ins.sync_info
                if esi is None:
                    extra.ins.sync_info = mybir.SyncInfo(on_update=[], on_wait=[w])
                else:
                    esi.on_wait = [w]
        self.nc.all_engine_barrier()
        assert self.sems is not None
        popped = self.nc._tile_sem_poison_stack.pop()
        assert popped is self._sem_poison
        self.nc.clear_and_free_semaphores(list(self.sems.allocated().values()))
        self.nc.all_engine_barrier()

    tile.TileContext._drain_and_barrier = _patched_dab
    tile.TileContext._drain_split_patched = True


def build_program():
    import concourse.bass as bass
    import concourse.mybir as mybir
    import concourse.tile as tile
    from concourse import bacc

    dt = mybir.dt
    AF = mybir.ActivationFunctionType
    Alu = mybir.AluOpType

    nc = bacc.Bacc()

    xT = nc.dram_tensor("xT", [K, T], dt.bfloat16, kind="ExternalInput")
    wq = nc.dram_tensor("wq", [K, 256], dt.bfloat16, kind="ExternalInput")
    wk = nc.dram_tensor("wk", [K, 256], dt.bfloat16, kind="ExternalInput")
    wv = nc.dram_tensor("wv", [K, 256], dt.bfloat16, kind="ExternalInput")
    wo = nc.dram_tensor("wo", [256, K], dt.bfloat16, kind="ExternalInput")
    out = nc.dram_tensor("out", [T, K], dt.float32, kind="ExternalOutput")

    with tile.TileContext(nc) as tc:
        with (
            tc.tile_pool(name="xt", bufs=KT) as xt_pool,
            tc.tile_pool(name="w", bufs=3) as w_pool,
            tc.tile_pool(name="wo", bufs=2) as wo_pool,
            tc.tile_pool(name="qk", bufs=4) as qk_pool,
            tc.tile_pool(name="v", bufs=1) as v_pool,
            tc.tile_pool(name="es", bufs=6) as es_pool,
            tc.tile_pool(name="acc", bufs=4) as acc_pool,
            tc.tile_pool(name="dd", bufs=2) as dd_pool,
            tc.tile_pool(name="yt", bufs=2) as yt_pool,
            tc.tile_pool(name="osb", bufs=2) as osb_pool,
            tc.tile_pool(name="st", bufs=2, space="PSUM") as st_pool,
            tc.tile_pool(name="av", bufs=2, space="PSUM") as av_pool,
            tc.tile_pool(name="pj", bufs=2, space="PSUM") as pj_pool,
        ):
            # ---------------- loads ----------------
            w_sb = {}
            for name, dram in (("q", wq), ("k", wk), ("v", wv)):
                t = w_pool.tile([128, KT * 256], dt.bfloat16, tag="w", name=f"w_{name}")
                nc.sync.dma_start(
                    t[:].rearrange("p (a c) -> p a c", a=KT),
                    dram.rearrange("(a p) c -> p a c", p=128),
                )
                w_sb[name] = t

            xt = []
            for a in range(KT):
                t = xt_pool.tile([128, T], dt.bfloat16, tag="xt", name=f"xt_{a}")
                nc.sync.dma_start(t[:], xT[a * 128 : (a + 1) * 128, :])
                xt.append(t)

            wo_sb = []
            for i in range(2):
                t = wo_pool.tile([128, K], dt.bfloat16, tag="wo", name=f"wo_{i}")
                nc.sync.dma_start(t[:], wo[i * 128 : (i + 1) * 128, :])
                wo_sb.append(t)

            # selector matrices: broadcast 1/D rows (at partitions 0/32/64/96)
            # over the 64-partition bands of the head pair, per 512-chunk c.
            sel = []
            for c in range(2):
                s = v_pool.tile([128, 128], dt.bfloat16, tag=f"sel{c}", name=f"sel_{c}")
                nc.vector.memset(s[:], 0.0)
                nc.vector.memset(s[32 * c : 32 * c + 1, 0:64], 1.0)
                nc.vector.memset(s[64 + 32 * c : 64 + 32 * c + 1, 64:128], 1.0)
                sel.append(s)

            ones = v_pool.tile([128, 1], dt.float16, tag="ones", name="ones")
            nc.vector.memset(ones[:], 1.0)

            # V in [u, s] orientation: [128 u, 16 ub * 256 (4 heads x 64)]
            v_sb = v_pool.tile([128, TB * 256], dt.float16, tag="v", name="v_sb")

            qt_sb = [None, None]
            kt_sb = [None, None]
            yt_sb = [
                yt_pool.tile([128, T], dt.bfloat16, tag="yt", name=f"yt_{hp}")
                for hp in range(2)
            ]

            # ---------------- projection chunks (fillers) ----------------
            def v_chunk(ub):
                """V[u-block ub, 256] = sum_a xt[a][:,ub].T @ wv[a]."""
                pv = pj_pool.tile([128, 512], dt.float32, tag="pj", name=f"pv_{ub}")
                for a in range(KT):
                    nc.tensor.matmul(
                        pv[:, 0:256],
                        xt[a][:, ub * 128 : (ub + 1) * 128],
                        w_sb["v"][:, a * 256 : (a + 1) * 256],
                        start=(a == 0),
                        stop=(a == KT - 1),
                    )
                nc.vector.tensor_copy(
                    v_sb[:, ub * 256 : (ub + 1) * 256], pv[:, 0:256]
                )

            def qk_chunk(hp, which, cp):
                """q or k projection for head pair hp, column pair cp
                (2 x 512 t-columns); stationary weight slice shared across
                the c pair, accumulating a=0..7 into two pj banks."""
                if which == "q" and qt_sb[hp] is None:
                    qt_sb[hp] = qk_pool.tile(
                        [128, T], dt.bfloat16, tag="qk", name=f"qt_{hp}"
                    )
                if which == "k" and kt_sb[hp] is None:
                    kt_sb[hp] = qk_pool.tile(
                        [128, T], dt.bfloat16, tag="qk", name=f"kt_{hp}"
                    )
                dst = qt_sb[hp] if which == "q" else kt_sb[hp]
                ps = [
                    pj_pool.tile(
                        [128, 512], dt.float32, tag="pj", name=f"p{which}_{hp}_{cp}_{i}"
                    )
                    for i in range(2)
                ]
                for a in range(KT):
                    for i in range(2):
                        nc.tensor.matmul(
                            ps[i][:],
                            w_sb[which][:, a * 256 + hp * 128 : a * 256 + hp * 128 + 128],
                            xt[a][:, cp * 1024 + i * 512 : cp * 1024 + (i + 1) * 512],
                            start=(a == 0),
                            stop=(a == KT - 1),
                        )
                for i in range(2):
                    tsl = slice(cp * 1024 + i * 512, cp * 1024 + (i + 1) * 512)
                    nc.vector.tensor_copy(dst[:, tsl], ps[i][:])

            def out_chunk(tb):
                """output projection for t-block tb; DVE evac; DMA out."""
                osb = osb_pool.tile([128, K], dt.float32, tag="osb", name=f"osb_{tb}")
                pso = [
                    pj_pool.tile([128, 512], dt.float32, tag="pj", name=f"po_{tb}_{i}")
                    for i in range(2)
                ]
                for hp in range(2):
                    for oc in range(2):
                        nc.tensor.matmul(
                            pso[oc][:],
                            yt_sb[hp][:, tb * 128 : (tb + 1) * 128],
                            wo_sb[hp][:, oc * 512 : (oc + 1) * 512],
                            start=(hp == 0),
                            stop=(hp == 1),
                        )
                for oc in range(2):
                    nc.vector.tensor_copy(osb[:, oc * 512 : (oc + 1) * 512], pso[oc][:])
                nc.sync.dma_start(out[tb * 128 : (tb + 1) * 128, :], osb[:])

            # ---------------- attention phase ----------------
            def attention_phase(hp, th, fillers):
                """One (head pair, t-half) block.  fillers: callables emitting
                one chunk each, pumped one per ub iteration.  Returns a
                finalize closure (normalization) to pump into the NEXT phase
                (it must be emitted before that phase's first AV matmul)."""
                t0 = th * 1024
                qt, kt = qt_sb[hp], kt_sb[hp]
                av = [
                    av_pool.tile(
                        [128, 512], dt.float32, tag="av", name=f"av_{hp}_{th}_{c}"
                    )
                    for c in range(2)
                ]
                acc = {}

                def scores(ub):
                    sts = [
                        st_pool.tile(
                            [128, 1024], dt.float32, tag="st",
                            name=f"st_{hp}_{th}_{ub}_{hl}",
                        )
                        for hl in range(2)
                    ]
                    # adjacent row-tile pairs -> concurrent execution
                    for c in range(2):
                        for hl in range(2):
                            nc.tensor.matmul(
                                sts[hl][:, c * 512 : (c + 1) * 512],
                                kt[hl * 64 : (hl + 1) * 64, ub * 128 : (ub + 1) * 128],
                                qt[hl * 64 : (hl + 1) * 64, t0 + c * 512 : t0 + (c + 1) * 512],
                                start=True,
                                stop=True,
                            )
                    return sts

                def expify(ub, sts):
                    ess = []
                    for hl in range(2):
                        e = es_pool.tile(
                            [128, 1024], dt.float16, tag="es",
                            name=f"e_{hp}_{th}_{ub}_{hl}",
                        )
                        nc.scalar.activation(e[:], sts[hl][:], AF.Exp)
                        ess.append(e)
                    return ess

                def avmm(ub, ess):
                    for c in range(2):
                        for hl in range(2):
                            vcol = ub * 256 + (2 * hp + hl) * 64
                            nc.tensor.matmul(
                                av[c][hl * 64 : (hl + 1) * 64, :],
                                v_sb[:, vcol : vcol + 64],
                                ess[hl][:, c * 512 : (c + 1) * 512],
                                start=(ub == 0),
                                stop=(ub == TB - 1),
                                skip_group_check=True,
                            )

                def accum(ub, ess):
                    for hl in range(2):
                        eng = nc.gpsimd if (_GPS_ACC and hl == 1) else nc.vector
                        if ub == 0:
                            a0 = acc_pool.tile(
                                [128, 1024], dt.float16, tag="acc",
                                name=f"acc_{hp}_{th}_{hl}_0",
                            )
                            eng.tensor_copy(a0[:], ess[hl][:])
                            acc[hl] = a0
                        else:
                            nxt = acc_pool.tile(
                                [128, 1024], dt.float16, tag="acc",
                                name=f"acc_{hp}_{th}_{hl}_{ub}",
                            )
                            eng.tensor_tensor(
                                nxt[:], acc[hl][:], ess[hl][:], op=Alu.add
                            )
                            acc[hl] = nxt

                # software pipeline: AV lags scores by one ub
                fi = 0
                prev = None
                for ub in range(TB):
                    sts = scores(ub)
                    ess = expify(ub, sts)
                    if prev is not None:
                        avmm(prev[0], prev[1])
                        accum(prev[0], prev[1])
                    if fi < len(fillers):
                        fillers[fi]()
                        fi += 1
                    prev = (ub, ess)
                avmm(prev[0], prev[1])
                accum(prev[0], prev[1])
                while fi < len(fillers):
                    fillers[fi]()
                    fi += 1

                # denominators: ones-matmul partition reduction of acc
                pjd = [
                    pj_pool.tile(
                        [128, 512], dt.float32, tag="pj", name=f"pjd_{hp}_{th}_{c}"
                    )
                    for c in range(2)
                ]
                for c in range(2):
                    for hl in range(2):
                        nc.tensor.matmul(
                            pjd[c][hl * 64 : hl * 64 + 1, :],
                            ones[:],
                            acc[hl][:, c * 512 : (c + 1) * 512],
                            start=True,
                            stop=True,
                            skip_group_check=True,
                        )
                drows = dd_pool.tile(
                    [128, 512], dt.float32, tag="drows", name=f"drows_{hp}_{th}"
                )
                nc.gpsimd.memset(drows[:], 1.0)
                for c in range(2):
                    for hl in range(2):
                        p = hl * 64 + c * 32
                        nc.vector.tensor_copy(
                            drows[p : p + 1, :], pjd[c][hl * 64 : hl * 64 + 1, :]
                        )
                # 1/D = exp(-ln(D)) on ACT (same table set as softmax Exp)
                dln = dd_pool.tile(
                    [128, 512], dt.float32, tag="dln", name=f"dln_{hp}_{th}"
                )
                nc.scalar.activation(dln[:], drows[:], AF.Ln)
                dinvb = dd_pool.tile(
                    [128, 512], dt.bfloat16, tag="dinvb", name=f"dinvb_{hp}_{th}"
                )
                nc.scalar.activation(dinvb[:], dln[:], AF.Exp, scale=-1.0)

                def finalize():
                    # broadcast 1/D over partition bands via sel matmul,
                    # then yt = av * (1/D)
                    for c in range(2):
                        dbp = pj_pool.tile(
                            [128, 512], dt.float32, tag="pj", name=f"dbp_{hp}_{th}_{c}"
                        )
                        nc.tensor.matmul(
                            dbp[:], sel[c][:], dinvb[:], start=True, stop=True
                        )
                        dbs = dd_pool.tile(
                            [128, 512], dt.float32, tag="dbs", name=f"dbs_{hp}_{th}_{c}"
                        )
                        nc.vector.tensor_copy(dbs[:], dbp[:])
                        nc.vector.tensor_tensor(
                            yt_sb[hp][:, t0 + c * 512 : t0 + (c + 1) * 512],
                            av[c][:],
                            dbs[:],
                            op=Alu.mult,
                        )

                return finalize

            # ---------------- schedule ----------------
            for which in ("q", "k"):
                for cp in range(2):
                    qk_chunk(0, which, cp)
            v_chunk(0)
            v_chunk(1)

            fillers_a = [(lambda u=ub: v_chunk(u)) for ub in range(2, TB)] + [
                (lambda w=w, c=c: qk_chunk(1, w, c))
                for w in ("q", "k")
                for c in range(2)
            ]
            fin_a = attention_phase(0, 0, fillers_a)

            fin_b = attention_phase(1, 0, [fin_a])

            fillers_c = [fin_b] + [(lambda t=tb: out_chunk(t)) for tb in range(0, 8)]
            fin_c = attention_phase(0, 1, fillers_c)

            fin_d = attention_phase(1, 1, [fin_c])
            fin_d()

            for tb in range(8, 16):
                out_chunk(tb)

    nc.finalize()
    return nc


def _prepare_in_maps(x, Wq, Wk, Wv, Wo):
    scale = 1.0 / math.sqrt(K)
    xT = [np.ascontiguousarray(x[b].T).astype(_BF16) for b in range(B)]
    in_maps = []
    for c in range(NCORES):
        b = c // 4
        g = c % 4
        sl = slice(g * 256, (g + 1) * 256)
        in_maps.append(
            {
                "xT": xT[b],
                "wq": np.ascontiguousarray((Wq[sl, :].astype(np.float64) * scale).T).astype(_BF16),
                "wk": np.ascontiguousarray(Wk[sl, :].T).astype(_BF16),
                "wv": np.ascontiguousarray(Wv[sl, :].T).astype(_BF16),
                "wo": np.ascontiguousarray(Wo[:, sl].T).astype(_BF16),
            }
        )
    return in_maps


def _gather(results, bo):
    out = np.zeros((B, T, K), dtype=np.float32)
    for b in range(B):
        acc = np.zeros((T, K), dtype=np.float32)
        for g in range(4):
            acc += results[b * 4 + g]["out"].astype(np.float32)
        out[b] = acc + bo.astype(np.float32)[None, :]
    return out


def _maybe_enable_ldw_opt():
    import os
    import concourse.bass_utils as bu

    if os.environ.get("LDWOPT", "0") != "1":
        return
    if getattr(bu, "_ldwopt_patched", False):
        return
    orig = bu.run_command

    def patched(argv, **kw):
        argv = [
            "--enable-ldw-opt=true" if a == "--enable-ldw-opt=false" else a
            for a in argv
        ]
        return orig(argv, **kw)

    bu.run_command = patched
    bu._ldwopt_patched = True


def run(x, Wq, Wk, Wv, Wo, bo, trace=False, tmpdir=None):
    from concourse.bass_utils import run_bass_kernel_spmd

    _maybe_enable_ldw_opt()
    _install_drain_split_patch()

    nc = build_program()
    in_maps = _prepare_in_maps(
        np.asarray(x), np.asarray(Wq), np.asarray(Wk), np.asarray(Wv), np.asarray(Wo)
    )
    res = run_bass_kernel_spmd(
        nc, in_maps, list(range(NCORES)), trace=trace, tmpdir=tmpdir
    )
    out = _gather(res.results, np.asarray(bo))
    return out, res


def kernel(x, Wq, Wk, Wv, Wo, bo):
    out, _ = run(x, Wq, Wk, Wv, Wo, bo, trace=False)
    return out


# revision 5
# speedup vs baseline: 1.4069x; 1.0312x over previous
"""Multi-head attention (16 heads, d_model=1024, T=2048, B=2) on 8 trn2 NeuronCores.

Sharding: core c -> batch c//4, head-group c%4 (4 heads of 64 dims each).
Each core computes q/k/v projections for its 4 heads on its batch, full
softmax attention for those heads, and a partial output projection
(row-parallel Wo).  Host sums the 4 partials per batch and adds the bias.

v2 design (vs baseline):
  - score matmuls (contract=64) emitted as adjacent h0/h64 row-tile pairs
    -> run concurrently on the two 64x128 PE tiles (~2x).
  - attn*V matmuls col-tiled: h0 -> psum partitions 0-63, h1 -> 64-127 of
    the same bank (~2x, and AV psum shrinks to 2 banks).
  - softmax denominators: fp16 running accumulation of the exp tiles on
    DVE/GpSimd, then a single ones-vector matmul per (head, chunk) reduces
    the 128 partitions; 1/D via ACT Ln + Exp(-x) (same act table set as
    the softmax Exp, so no table reloads).
  - V projection computed directly in [u, s] orientation (stationary xT
    block, moving Wv) - no PE transposes.
  - static software pipeline: projection / output chunks are interleaved
    into the ACT(exp)-bound attention phases as fillers; each phase's
    normalization is emitted as the first filler of the next phase so the
    PE never waits on the 1/D chain.
PSUM budget: st 2x[128,1024] (4 banks) + av 2x[128,512] (2) + pj 2 (2).
"""

import math
import os as _os

import numpy as np
import ml_dtypes

B = 2
T = 2048
K = 1024
H = 16
S = K // H  # 64
NCORES = 8
TB = T // 128  # 16 u-blocks
KT = K // 128  # 8 contraction tiles

_BF16 = ml_dtypes.bfloat16
_GPS_ACC = _os.environ.get("GPS_ACC", "0") == "1"


def _install_drain_split_patch():
    """walrus in this container rejects >1 sync-wait on the final tile drain;
    split the waits one-per-drain-instruction (all before the end barrier)."""
    import concourse.tile as tile
    import concourse.mybir as mybir
    from concourse.vector_clock import ScopedClock

    if getattr(tile.TileContext, "_drain_split_patched", False):
        return

    def _patched_dab(self, tick_clock, wait_clock):
        drain_inst = self.nc.sync.drain()
        wait_clock.add_sem_waits(
            drain_inst.ins, ScopedClock({None: tick_clock.global_clock})
        )
        si = drain_inst.ins.sync_info
        waits = list(si.on_wait) if si is not None else []
        if len(waits) > 1:
            si.on_wait = waits[:1]
            for w in waits[1:]:
                extra = self.nc.sync.drain()
                esi = extra.ins.sync_info
                if esi is None:
                    extra.ins.sync_info = mybir.SyncInfo(on_update=[], on_wait=[w])
                else:
                    esi.on_wait = [w]
        self.nc.all_engine_barrier()
        assert self.sems is not None
        popped = self.nc._tile_sem_poison_stack.pop()
        assert popped is self._sem_poison
        self.nc.clear_and_free_semaphores(list(self.sems.allocated().values()))
        self.nc.all_engine_barrier()

    tile.TileContext._drain_and_barrier = _patched_dab
    tile.TileContext._drain_split_patched = True


def build_program():
    import concourse.bass as bass
    import concourse.mybir as mybir
    import concourse.tile as tile
    from concourse import bacc

    dt = mybir.dt
    AF = mybir.ActivationFunctionType
    Alu = mybir.AluOpType

    nc = bacc.Bacc()

    xT = nc.dram_tensor("xT", [K, T], dt.bfloat16, kind="ExternalInput")
    wq = nc.dram_tensor("wq", [K, 256], dt.bfloat16, kind="ExternalInput")
    wk = nc.dram_tensor("wk", [K, 256], dt.bfloat16, kind="ExternalInput")
    wv = nc.dram_tensor("wv", [K, 256], dt.bfloat16, kind="ExternalInput")
    wo = nc.dram_tensor("wo", [256, K], dt.bfloat16, kind="ExternalInput")
    out = nc.dram_tensor("out", [T, K], dt.float32, kind="ExternalOutput")

    with tile.TileContext(nc) as tc:
        with (
            tc.tile_pool(name="xt", bufs=KT) as xt_pool,
            tc.tile_pool(name="w", bufs=3) as w_pool,
            tc.tile_pool(name="wo", bufs=2) as wo_pool,
            tc.tile_pool(name="qk", bufs=4) as qk_pool,
            tc.tile_pool(name="v", bufs=1) as v_pool,
            tc.tile_pool(name="es", bufs=6) as es_pool,
            tc.tile_pool(name="acc", bufs=4) as acc_pool,
            tc.tile_pool(name="dd", bufs=2) as dd_pool,
            tc.tile_pool(name="yt", bufs=2) as yt_pool,
            tc.tile_pool(name="osb", bufs=2) as osb_pool,
            tc.tile_pool(name="st", bufs=2, space="PSUM") as st_pool,
            tc.tile_pool(name="av", bufs=2, space="PSUM") as av_pool,
            tc.tile_pool(name="pj", bufs=2, space="PSUM") as pj_pool,
        ):
            # ---------------- loads ----------------
            w_sb = {}
            for name, dram in (("q", wq), ("k", wk), ("v", wv)):
                t = w_pool.tile([128, KT * 256], dt.bfloat16, tag="w", name=f"w_{name}")
                nc.sync.dma_start(
                    t[:].rearrange("p (a c) -> p a c", a=KT),
                    dram.rearrange("(a p) c -> p a c", p=128),
                )
                w_sb[name] = t

            xt = []
            for a in range(KT):
                t = xt_pool.tile([128, T], dt.bfloat16, tag="xt", name=f"xt_{a}")
                nc.sync.dma_start(t[:], xT[a * 128 : (a + 1) * 128, :])
                xt.append(t)

            wo_sb = []
            for i in range(2):
                t = wo_pool.tile([128, K], dt.bfloat16, tag="wo", name=f"wo_{i}")
                nc.sync.dma_start(t[:], wo[i * 128 : (i + 1) * 128, :])
                wo_sb.append(t)

            # selector matrices: broadcast 1/D rows (at partitions 0/32/64/96)
            # over the 64-partition bands of the head pair, per 512-chunk c.
            sel = []
            for c in range(2):
                s = v_pool.tile([128, 128], dt.bfloat16, tag=f"sel{c}", name=f"sel_{c}")
                nc.vector.memset(s[:], 0.0)
                nc.vector.memset(s[32 * c : 32 * c + 1, 0:64], 1.0)
                nc.vector.memset(s[64 + 32 * c : 64 + 32 * c + 1, 64:128], 1.0)
                sel.append(s)

            ones = v_pool.tile([128, 1], dt.float16, tag="ones", name="ones")
            nc.vector.memset(ones[:], 1.0)

            # V in [u, s] orientation: [128 u, 16 ub * 256 (4 heads x 64)]
            v_sb = v_pool.tile([128, TB * 256], dt.float16, tag="v", name="v_sb")

            qt_sb = [None, None]
            kt_sb = [None, None]
            yt_sb = [
                yt_pool.tile([128, T], dt.bfloat16, tag="yt", name=f"yt_{hp}")
                for hp in range(2)
            ]

            # ---------------- projection chunks (fillers) ----------------
            def v_chunk(ub):
                """V[u-block ub, 256] = sum_a xt[a][:,ub].T @ wv[a]."""
                pv = pj_pool.tile([128, 512], dt.float32, tag="pj", name=f"pv_{ub}")
                for a in range(KT):
                    nc.tensor.matmul(
                        pv[:, 0:256],
                        xt[a][:, ub * 128 : (ub + 1) * 128],
                        w_sb["v"][:, a * 256 : (a + 1) * 256],
                        start=(a == 0),
                        stop=(a == KT - 1),
                    )
                nc.vector.tensor_copy(
                    v_sb[:, ub * 256 : (ub + 1) * 256], pv[:, 0:256]
                )

            def qk_chunk(hp, which, cp):
                """q or k projection for head pair hp, column pair cp
                (2 x 512 t-columns); stationary weight slice shared across
                the c pair, accumulating a=0..7 into two pj banks."""
                if which == "q" and qt_sb[hp] is None:
                    qt_sb[hp] = qk_pool.tile(
                        [128, T], dt.bfloat16, tag="qk", name=f"qt_{hp}"
                    )
                if which == "k" and kt_sb[hp] is None:
                    kt_sb[hp] = qk_pool.tile(
                        [128, T], dt.bfloat16, tag="qk", name=f"kt_{hp}"
                    )
                dst = qt_sb[hp] if which == "q" else kt_sb[hp]
                ps = [
                    pj_pool.tile(
                        [128, 512], dt.float32, tag="pj", name=f"p{which}_{hp}_{cp}_{i}"
                    )
                    for i in range(2)
                ]
                for a in range(KT):
                    for i in range(2):
                        nc.tensor.matmul(
                            ps[i][:],
                            w_sb[which][:, a * 256 + hp * 128 : a * 256 + hp * 128 + 128],
                            xt[a][:, cp * 1024 + i * 512 : cp * 1024 + (i + 1) * 512],
                            start=(a == 0),
                            stop=(a == KT - 1),
                        )
                for i in range(2):
                    tsl = slice(cp * 1024 + i * 512, cp * 1024 + (i + 1) * 512)
                    nc.vector.tensor_copy(dst[:, tsl], ps[i][:])

            def out_chunk(tb):
                """output projection for t-block tb; DVE evac; DMA out."""
                osb = osb_pool.tile([128, K], dt.float32, tag="osb", name=f"osb_{tb}")
                pso = [
                    pj_pool.tile([128, 512], dt.float32, tag="pj", name=f"po_{tb}_{i}")
                    for i in range(2)
                ]
                for hp in range(2):
                    for oc in range(2):
                        nc.tensor.matmul(
                            pso[oc][:],
                            yt_sb[hp][:, tb * 128 : (tb + 1) * 128],
                            wo_sb[hp][:, oc * 512 : (oc + 1) * 512],
                            start=(hp == 0),
                            stop=(hp == 1),
                        )
                for oc in range(2):
                    nc.vector.tensor_copy(osb[:, oc * 512 : (oc + 1) * 512], pso[oc][:])
                nc.sync.dma_start(out[tb * 128 : (tb + 1) * 128, :], osb[:])

            # ---------------- attention phase ----------------
            def attention_phase(hp, th, fillers):
                """One (head pair, t-half) block.  fillers: callables emitting
                one chunk each, pumped one per ub iteration.  Returns a
                finalize closure (normalization) to pump into the NEXT phase
                (it must be emitted before that phase's first AV matmul)."""
                t0 = th * 1024
                qt, kt = qt_sb[hp], kt_sb[hp]
                av = [
                    av_pool.tile(
                        [128, 512], dt.float32, tag="av", name=f"av_{hp}_{th}_{c}"
                    )
                    for c in range(2)
                ]
                acc = {}

                def scores(ub):
                    sts = [
                        st_pool.tile(
                            [128, 1024], dt.float32, tag="st",
                            name=f"st_{hp}_{th}_{ub}_{hl}",
                        )
                        for hl in range(2)
                    ]
                    # adjacent row-tile pairs -> concurrent execution
                    for c in range(2):
                        for hl in range(2):
                            nc.tensor.matmul(
                                sts[hl][:, c * 512 : (c + 1) * 512],
                                kt[hl * 64 : (hl + 1) * 64, ub * 128 : (ub + 1) * 128],
                                qt[hl * 64 : (hl + 1) * 64, t0 + c * 512 : t0 + (c + 1) * 512],
                                start=True,
                                stop=True,
                            )
                    return sts

                def expify(ub, sts):
                    ess = []
                    for hl in range(2):
                        e = es_pool.tile(
                            [128, 1024], dt.float16, tag="es",
                            name=f"e_{hp}_{th}_{ub}_{hl}",
                        )
                        nc.scalar.activation(e[:], sts[hl][:], AF.Exp)
                        ess.append(e)
                    return ess

                def avmm(ub, ess):
                    for c in range(2):
                        for hl in range(2):
                            vcol = ub * 256 + (2 * hp + hl) * 64
                            nc.tensor.matmul(
                                av[c][hl * 64 : (hl + 1) * 64, :],
                                v_sb[:, vcol : vcol + 64],
                                ess[hl][:, c * 512 : (c + 1) * 512],
                                start=(ub == 0),
                                stop=(ub == TB - 1),
                                skip_group_check=True,
                            )

                def accum(ub, ess):
                    for hl in range(2):
                        eng = nc.gpsimd if (_GPS_ACC and hl == 1) else nc.vector
                        if ub == 0:
                            a0 = acc_pool.tile(
                                [128, 1024], dt.float16, tag="acc",
                                name=f"acc_{hp}_{th}_{hl}_0",
                            )
                            eng.tensor_copy(a0[:], ess[hl][:])
                            acc[hl] = a0
                        else:
                            nxt = acc_pool.tile(
                                [128, 1024], dt.float16, tag="acc",
                                name=f"acc_{hp}_{th}_{hl}_{ub}",
                            )
                            eng.tensor_tensor(
                                nxt[:], acc[hl][:], ess[hl][:], op=Alu.add
                            )
                            acc[hl] = nxt

                # software pipeline: AV lags scores by one ub
                fi = 0
                prev = None
                for ub in range(TB):
                    sts = scores(ub)
                    ess = expify(ub, sts)
                    if prev is not None:
                        avmm(prev[0], prev[1])
                        accum(prev[0], prev[1])
                    if fi < len(fillers):
                        fillers[fi]()
                        fi += 1
                    prev = (ub, ess)
                avmm(prev[0], prev[1])
                accum(prev[0], prev[1])
                while fi < len(fillers):
                    fillers[fi]()
                    fi += 1

                # denominators: ones-matmul partition reduction of acc
                pjd = [
                    pj_pool.tile(
                        [128, 512], dt.float32, tag="pj", name=f"pjd_{hp}_{th}_{c}"
                    )
                    for c in range(2)
                ]
                for c in range(2):
                    for hl in range(2):
                        nc.tensor.matmul(
                            pjd[c][hl * 64 : hl * 64 + 1, :],
                            ones[:],
                            acc[hl][:, c * 512 : (c + 1) * 512],
                            start=True,
                            stop=True,
                            skip_group_check=True,
                        )
                drows = dd_pool.tile(
                    [128, 512], dt.float32, tag="drows", name=f"drows_{hp}_{th}"
                )
                nc.gpsimd.memset(drows[:], 1.0)
                for c in range(2):
                    for hl in range(2):
                        p = hl * 64 + c * 32
                        nc.vector.tensor_copy(
                            drows[p : p + 1, :], pjd[c][hl * 64 : hl * 64 + 1, :]
                        )
                # 1/D via single-op approx reciprocal (~18 bits, plenty)
                dinv = dd_pool.tile(
                    [128, 512], dt.float32, tag="dln", name=f"dinv_{hp}_{th}"
                )
                nc.vector.reciprocal_approx_fast(dinv[:], drows[:])
                dinvb = dd_pool.tile(
                    [128, 512], dt.bfloat16, tag="dinvb", name=f"dinvb_{hp}_{th}"
                )
                nc.vector.tensor_copy(dinvb[:], dinv[:])

                def finalize():
                    # broadcast 1/D over partition bands via sel matmul,
                    # then yt = av * (1/D)
                    for c in range(2):
                        dbp = pj_pool.tile(
                            [128, 512], dt.float32, tag="pj", name=f"dbp_{hp}_{th}_{c}"
                        )
                        nc.tensor.matmul(
                            dbp[:], sel[c][:], dinvb[:], start=True, stop=True
                        )
                        dbs = dd_pool.tile(
                            [128, 512], dt.float32, tag="dbs", name=f"dbs_{hp}_{th}_{c}"
                        )
                        nc.vector.tensor_copy(dbs[:], dbp[:])
                        nc.vector.tensor_tensor(
                            yt_sb[hp][:, t0 + c * 512 : t0 + (c + 1) * 512],
                            av[c][:],
                            dbs[:],
                            op=Alu.mult,
                        )

                return finalize

            # ---------------- schedule ----------------
            for which in ("q", "k"):
                for cp in range(2):
                    qk_chunk(0, which, cp)
            v_chunk(0)
            v_chunk(1)

            fillers_a = [(lambda u=ub: v_chunk(u)) for ub in range(2, TB)] + [
                (lambda w=w, c=c: qk_chunk(1, w, c))
                for w in ("q", "k")
                for c in range(2)
            ]
            fin_a = attention_phase(0, 0, fillers_a)

            fin_b = attention_phase(1, 0, [fin_a])

            fillers_c = [fin_b] + [(lambda t=tb: out_chunk(t)) for tb in range(0, 8)]
            fin_c = attention_phase(0, 1, fillers_c)

            fin_d = attention_phase(1, 1, [fin_c])
            fin_d()

            for tb in range(8, 16):
                out_chunk(tb)

    nc.finalize()
    return nc


def _prepare_in_maps(x, Wq, Wk, Wv, Wo):
    scale = 1.0 / math.sqrt(K)
    xT = [np.ascontiguousarray(x[b].T).astype(_BF16) for b in range(B)]
    in_maps = []
    for c in range(NCORES):
        b = c // 4
        g = c % 4
        sl = slice(g * 256, (g + 1) * 256)
        in_maps.append(
            {
                "xT": xT[b],
                "wq": np.ascontiguousarray((Wq[sl, :].astype(np.float64) * scale).T).astype(_BF16),
                "wk": np.ascontiguousarray(Wk[sl, :].T).astype(_BF16),
                "wv": np.ascontiguousarray(Wv[sl, :].T).astype(_BF16),
                "wo": np.ascontiguousarray(Wo[:, sl].T).astype(_BF16),
            }
        )
    return in_maps


def _gather(results, bo):
    out = np.zeros((B, T, K), dtype=np.float32)
    for b in range(B):
        acc = np.zeros((T, K), dtype=np.float32)
        for g in range(4):
            acc += results[b * 4 + g]["out"].astype(np.float32)
        out[b] = acc + bo.astype(np.float32)[None, :]
    return out


def _maybe_enable_ldw_opt():
    import os
    import concourse.bass_utils as bu

    if os.environ.get("LDWOPT", "0") != "1":
        return
    if getattr(bu, "_ldwopt_patched", False):
        return
    orig = bu.run_command

    def patched(argv, **kw):
        argv = [
            "--enable-ldw-opt=true" if a == "--enable-ldw-opt=false" else a
            for a in argv
        ]
        return orig(argv, **kw)

    bu.run_command = patched
    bu._ldwopt_patched = True


def run(x, Wq, Wk, Wv, Wo, bo, trace=False, tmpdir=None):
    from concourse.bass_utils import run_bass_kernel_spmd

    _maybe_enable_ldw_opt()
    _install_drain_split_patch()

    nc = build_program()
    in_maps = _prepare_in_maps(
        np.asarray(x), np.asarray(Wq), np.asarray(Wk), np.asarray(Wv), np.asarray(Wo)
    )
    res = run_bass_kernel_spmd(
        nc, in_maps, list(range(NCORES)), trace=trace, tmpdir=tmpdir
    )
    out = _gather(res.results, np.asarray(bo))
    return out, res


def kernel(x, Wq, Wk, Wv, Wo, bo):
    out, _ = run(x, Wq, Wk, Wv, Wo, bo, trace=False)
    return out


# revision 14
# speedup vs baseline: 1.5609x; 1.1095x over previous
"""Multi-head attention (16 heads, d_model=1024, T=2048, B=2) on 8 trn2 NeuronCores.

Sharding: core c -> batch c//4, head-group c%4 (4 heads of 64 dims each).
Each core computes q/k/v projections for its 4 heads on its batch, full
softmax attention for those heads, and a partial output projection
(row-parallel Wo).  Host sums the 4 partials per batch and adds the bias.

v2 design (vs baseline):
  - score matmuls (contract=64) emitted as adjacent h0/h64 row-tile pairs
    -> run concurrently on the two 64x128 PE tiles (~2x).
  - attn*V matmuls col-tiled: h0 -> psum partitions 0-63, h1 -> 64-127 of
    the same bank (~2x, and AV psum shrinks to 2 banks).
  - softmax denominators: fp16 running accumulation of the exp tiles on
    DVE/GpSimd, then a single ones-vector matmul per (head, chunk) reduces
    the 128 partitions; 1/D via ACT Ln + Exp(-x) (same act table set as
    the softmax Exp, so no table reloads).
  - V projection computed directly in [u, s] orientation (stationary xT
    block, moving Wv) - no PE transposes.
  - static software pipeline: projection / output chunks are interleaved
    into the ACT(exp)-bound attention phases as fillers; each phase's
    normalization is emitted as the first filler of the next phase so the
    PE never waits on the 1/D chain.
PSUM budget: st 2x[128,1024] (4 banks) + av 2x[128,512] (2) + pj 2 (2).
"""

import math
import os as _os

import numpy as np
import ml_dtypes

B = 2
T = 2048
K = 1024
H = 16
S = K // H  # 64
NCORES = 8
TB = T // 128  # 16 u-blocks
KT = K // 128  # 8 contraction tiles

_BF16 = ml_dtypes.bfloat16
_GPS_ACC = _os.environ.get("GPS_ACC", "0") == "1"


def _install_drain_split_patch():
    """walrus in this container rejects >1 sync-wait on the final tile drain;
    split the waits one-per-drain-instruction (all before the end barrier)."""
    import concourse.tile as tile
    import concourse.mybir as mybir
    from concourse.vector_clock import ScopedClock

    if getattr(tile.TileContext, "_drain_split_patched", False):
        return

    def _patched_dab(self, tick_clock, wait_clock):
        drain_inst = self.nc.sync.drain()
        wait_clock.add_sem_waits(
            drain_inst.ins, ScopedClock({None: tick_clock.global_clock})
        )
        si = drain_inst.ins.sync_info
        waits = list(si.on_wait) if si is not None else []
        if len(waits) > 1:
            si.on_wait = waits[:1]
            for w in waits[1:]:
                extra = self.nc.sync.drain()
                esi = extra.ins.sync_info
                if esi is None:
                    extra.ins.sync_info = mybir.SyncInfo(on_update=[], on_wait=[w])
                else:
                    esi.on_wait = [w]
        self.nc.all_engine_barrier()
        assert self.sems is not None
        popped = self.nc._tile_sem_poison_stack.pop()
        assert popped is self._sem_poison
        self.nc.clear_and_free_semaphores(list(self.sems.allocated().values()))
        self.nc.all_engine_barrier()

    tile.TileContext._drain_and_barrier = _patched_dab
    tile.TileContext._drain_split_patched = True


def build_program():
    import concourse.bass as bass
    import concourse.mybir as mybir
    import concourse.tile as tile
    from concourse import bacc

    dt = mybir.dt
    AF = mybir.ActivationFunctionType
    Alu = mybir.AluOpType

    nc = bacc.Bacc()

    xT = nc.dram_tensor("xT", [K, T], dt.bfloat16, kind="ExternalInput")
    wq = nc.dram_tensor("wq", [K, 256], dt.bfloat16, kind="ExternalInput")
    wk = nc.dram_tensor("wk", [K, 256], dt.bfloat16, kind="ExternalInput")
    wv = nc.dram_tensor("wv", [K, 256], dt.bfloat16, kind="ExternalInput")
    wo = nc.dram_tensor("wo", [256, K], dt.bfloat16, kind="ExternalInput")
    out = nc.dram_tensor("out", [T, K], dt.float32, kind="ExternalOutput")

    with tile.TileContext(nc) as tc:
        with (
            tc.tile_pool(name="xt", bufs=KT) as xt_pool,
            tc.tile_pool(name="w", bufs=3) as w_pool,
            tc.tile_pool(name="wo", bufs=2) as wo_pool,
            tc.tile_pool(name="qk", bufs=4) as qk_pool,
            tc.tile_pool(name="v", bufs=1) as v_pool,
            tc.tile_pool(name="es", bufs=6) as es_pool,
            tc.tile_pool(name="acc", bufs=4) as acc_pool,
            tc.tile_pool(name="dd", bufs=2) as dd_pool,
            tc.tile_pool(name="yt", bufs=2) as yt_pool,
            tc.tile_pool(name="osb", bufs=2) as osb_pool,
            tc.tile_pool(name="st", bufs=2, space="PSUM") as st_pool,
            tc.tile_pool(name="av", bufs=2, space="PSUM") as av_pool,
            tc.tile_pool(name="pj", bufs=2, space="PSUM") as pj_pool,
        ):
            # ---------------- loads ----------------
            w_sb = {}
            for name, dram in (("q", wq), ("k", wk), ("v", wv)):
                t = w_pool.tile([128, KT * 256], dt.bfloat16, tag="w", name=f"w_{name}")
                nc.sync.dma_start(
                    t[:].rearrange("p (a c) -> p a c", a=KT),
                    dram.rearrange("(a p) c -> p a c", p=128),
                )
                w_sb[name] = t

            xt = []
            for a in range(KT):
                t = xt_pool.tile([128, T], dt.bfloat16, tag="xt", name=f"xt_{a}")
                nc.sync.dma_start(t[:], xT[a * 128 : (a + 1) * 128, :])
                xt.append(t)

            wo_sb = []
            for i in range(2):
                t = wo_pool.tile([128, K], dt.bfloat16, tag="wo", name=f"wo_{i}")
                nc.sync.dma_start(t[:], wo[i * 128 : (i + 1) * 128, :])
                wo_sb.append(t)

            # selector matrices: broadcast 1/D rows (at partitions 0/32/64/96)
            # over the 64-partition bands of the head pair, per 512-chunk c.
            sel = []
            for c in range(2):
                s = v_pool.tile([128, 128], dt.bfloat16, tag=f"sel{c}", name=f"sel_{c}")
                nc.vector.memset(s[:], 0.0)
                nc.vector.memset(s[32 * c : 32 * c + 1, 0:64], 1.0)
                nc.vector.memset(s[64 + 32 * c : 64 + 32 * c + 1, 64:128], 1.0)
                sel.append(s)

            # denominator selectors: ones column at 0 (h0 -> out row 0) or
            # 64 (h1 -> out row 64), zero elsewhere -> (128,128)-mode reduce
            dsel = []
            for hl in range(2):
                s = v_pool.tile(
                    [128, 128], dt.float16, tag=f"dsel{hl}", name=f"dsel_{hl}"
                )
                nc.vector.memset(s[:], 0.0)
                nc.vector.memset(s[:, hl * 64 : hl * 64 + 1], 1.0)
                dsel.append(s)

            # V stationaries, zero-padded to M=128 per (ub, hp):
            # block layout [v_h0 (64) | zeros (64) | v_h1 (64)] of 192 cols so
            # h0 reads cols 0:128 = [v|0], h1 reads cols 64:192 = [0|v].
            VBLK = 192
            v_sb = v_pool.tile([128, TB * 2 * VBLK], dt.float16, tag="v", name="v_sb")
            nc.gpsimd.memset(v_sb[:], 0.0)

            qt_sb = [None, None]
            ktp_sb = {}  # (hp, hl) -> partition-padded K^T tile
            yt_sb = [
                yt_pool.tile([128, T], dt.bfloat16, tag="yt", name=f"yt_{hp}")
                for hp in range(2)
            ]

            # ---------------- projection chunks (fillers) ----------------
            def v_chunk(ub):
                """V[u-block ub, 256] = sum_a xt[a][:,ub].T @ wv[a], scattered
                into the zero-padded v_sb blocks (2 strided copies)."""
                pv = pj_pool.tile([128, 512], dt.float32, tag="pj", name=f"pv_{ub}")
                for a in range(KT):
                    nc.tensor.matmul(
                        pv[:, 0:256],
                        xt[a][:, ub * 128 : (ub + 1) * 128],
                        w_sb["v"][:, a * 256 : (a + 1) * 256],
                        start=(a == 0),
                        stop=(a == KT - 1),
                    )
                base = ub * 2 * VBLK
                win = v_sb[:, base : base + 2 * VBLK].rearrange(
                    "p (hp c) -> p hp c", hp=2
                )
                src = pv[:, 0:256].rearrange("p (hp h c) -> p hp h c", hp=2, h=2)
                # h0 of both head pairs -> block offsets {0, 192}+0
                nc.vector.tensor_copy(win[:, :, 0:64], src[:, :, 0, :])
                # h1 of both head pairs -> block offsets {0, 192}+128
                nc.vector.tensor_copy(win[:, :, 128:192], src[:, :, 1, :])

            def qk_chunk(hp, which, cp):
                """q or k projection for head pair hp, column pair cp
                (2 x 512 t-columns); stationary weight slice shared across
                the c pair, accumulating a=0..7 into two pj banks.
                K evacuates into two partition-padded tiles (other head's
                64 partitions zeroed) so score matmuls run at full 128
                contract in uniform (128,128) mode."""
                if which == "q" and qt_sb[hp] is None:
                    qt_sb[hp] = qk_pool.tile(
                        [128, T], dt.bfloat16, tag="qk", name=f"qt_{hp}"
                    )
                if which == "k" and (hp, 0) not in ktp_sb:
                    for hl in range(2):
                        t = qk_pool.tile(
                            [128, T], dt.bfloat16, tag=f"ktp{hl}", name=f"ktp_{hp}_{hl}"
                        )
                        nc.gpsimd.memset(t[(1 - hl) * 64 : (2 - hl) * 64, :], 0.0)
                        ktp_sb[(hp, hl)] = t
                ps = [
                    pj_pool.tile(
                        [128, 512], dt.float32, tag="pj", name=f"p{which}_{hp}_{cp}_{i}"
                    )
                    for i in range(2)
                ]
                for a in range(KT):
                    for i in range(2):
                        nc.tensor.matmul(
                            ps[i][:],
                            w_sb[which][:, a * 256 + hp * 128 : a * 256 + hp * 128 + 128],
                            xt[a][:, cp * 1024 + i * 512 : cp * 1024 + (i + 1) * 512],
                            start=(a == 0),
                            stop=(a == KT - 1),
                        )
                for i in range(2):
                    tsl = slice(cp * 1024 + i * 512, cp * 1024 + (i + 1) * 512)
                    if which == "q":
                        nc.vector.tensor_copy(qt_sb[hp][:, tsl], ps[i][:])
                    else:
                        for hl in range(2):
                            psl = slice(hl * 64, (hl + 1) * 64)
                            nc.vector.tensor_copy(
                                ktp_sb[(hp, hl)][psl, tsl], ps[i][psl, :]
                            )

            def out_chunk(tb):
                """output projection for t-block tb; DVE evac; DMA out."""
                osb = osb_pool.tile([128, K], dt.float32, tag="osb", name=f"osb_{tb}")
                pso = [
                    pj_pool.tile([128, 512], dt.float32, tag="pj", name=f"po_{tb}_{i}")
                    for i in range(2)
                ]
                for hp in range(2):
                    for oc in range(2):
                        nc.tensor.matmul(
                            pso[oc][:],
                            yt_sb[hp][:, tb * 128 : (tb + 1) * 128],
                            wo_sb[hp][:, oc * 512 : (oc + 1) * 512],
                            start=(hp == 0),
                            stop=(hp == 1),
                        )
                for oc in range(2):
                    nc.vector.tensor_copy(osb[:, oc * 512 : (oc + 1) * 512], pso[oc][:])
                nc.sync.dma_start(out[tb * 128 : (tb + 1) * 128, :], osb[:])

            # ---------------- attention phase ----------------
            def attention_phase(hp, th, fillers):
                """One (head pair, t-half) block.  fillers: callables emitting
                one chunk each, pumped one per ub iteration.  Returns a
                finalize closure (normalization) to pump into the NEXT phase
                (it must be emitted before that phase's first AV matmul)."""
                t0 = th * 1024
                qt = qt_sb[hp]
                av = [
                    av_pool.tile(
                        [128, 512], dt.float32, tag="av", name=f"av_{hp}_{th}_{c}"
                    )
                    for c in range(2)
                ]
                acc = {}

                def scores(ub):
                    sts = [
                        st_pool.tile(
                            [128, 1024], dt.float32, tag="st",
                            name=f"st_{hp}_{th}_{ub}_{hl}",
                        )
                        for hl in range(2)
                    ]
                    # full-contract (zero-padded) -> uniform (128,128) mode
                    for c in range(2):
                        for hl in range(2):
                            nc.tensor.matmul(
                                sts[hl][:, c * 512 : (c + 1) * 512],
                                ktp_sb[(hp, hl)][:, ub * 128 : (ub + 1) * 128],
                                qt[:, t0 + c * 512 : t0 + (c + 1) * 512],
                                start=True,
                                stop=True,
                            )
                    return sts

                def expify(ub, sts):
                    ess = []
                    for hl in range(2):
                        e = es_pool.tile(
                            [128, 1024], dt.float16, tag="es",
                            name=f"e_{hp}_{th}_{ub}_{hl}",
                        )
                        nc.scalar.activation(e[:], sts[hl][:], AF.Exp)
                        ess.append(e)
                    return ess

                def avmm(ub, ess):
                    # M=128 zero-padded stationaries; h0 and h1 accumulate
                    # additively into the same bank (pad rows contribute 0)
                    for c in range(2):
                        for hl in range(2):
                            vcol = ub * 2 * VBLK + hp * VBLK + hl * 64
                            nc.tensor.matmul(
                                av[c][:],
                                v_sb[:, vcol : vcol + 128],
                                ess[hl][:, c * 512 : (c + 1) * 512],
                                start=(ub == 0 and hl == 0),
                                stop=(ub == TB - 1 and hl == 1),
                                skip_group_check=True,
                            )

                def accum(ub, ess):
                    for hl in range(2):
                        eng = nc.gpsimd if (_GPS_ACC and hl == 1) else nc.vector
                        if ub == 0:
                            a0 = acc_pool.tile(
                                [128, 1024], dt.float16, tag="acc",
                                name=f"acc_{hp}_{th}_{hl}_0",
                            )
                            eng.tensor_copy(a0[:], ess[hl][:])
                            acc[hl] = a0
                        else:
                            nxt = acc_pool.tile(
                                [128, 1024], dt.float16, tag="acc",
                                name=f"acc_{hp}_{th}_{hl}_{ub}",
                            )
                            eng.tensor_tensor(
                                nxt[:], acc[hl][:], ess[hl][:], op=Alu.add
                            )
                            acc[hl] = nxt

                # software pipeline: AV lags scores by one ub
                fi = 0
                prev = None
                for ub in range(TB):
                    sts = scores(ub)
                    ess = expify(ub, sts)
                    if prev is not None:
                        avmm(prev[0], prev[1])
                        accum(prev[0], prev[1])
                    if fi < len(fillers):
                        fillers[fi]()
                        fi += 1
                    prev = (ub, ess)
                avmm(prev[0], prev[1])
                accum(prev[0], prev[1])
                while fi < len(fillers):
                    fillers[fi]()
                    fi += 1

                # denominators: padded-selector matmul partition reduction
                # (D_h0 -> out row 0, D_h1 -> row 64, same bank, same mode)
                pjd = [
                    pj_pool.tile(
                        [128, 512], dt.float32, tag="pj", name=f"pjd_{hp}_{th}_{c}"
                    )
                    for c in range(2)
                ]
                for c in range(2):
                    for hl in range(2):
                        nc.tensor.matmul(
                            pjd[c][:],
                            dsel[hl][:],
                            acc[hl][:, c * 512 : (c + 1) * 512],
                            start=(hl == 0),
                            stop=(hl == 1),
                            skip_group_check=True,
                        )
                drows = dd_pool.tile(
                    [128, 512], dt.float32, tag="drows", name=f"drows_{hp}_{th}"
                )
                nc.gpsimd.memset(drows[:], 1.0)
                for c in range(2):
                    for hl in range(2):
                        p = hl * 64 + c * 32
                        nc.vector.tensor_copy(
                            drows[p : p + 1, :], pjd[c][hl * 64 : hl * 64 + 1, :]
                        )
                # 1/D via single-op approx reciprocal (~18 bits, plenty)
                dinv = dd_pool.tile(
                    [128, 512], dt.float32, tag="dln", name=f"dinv_{hp}_{th}"
                )
                nc.vector.reciprocal_approx_fast(dinv[:], drows[:])
                dinvb = dd_pool.tile(
                    [128, 512], dt.bfloat16, tag="dinvb", name=f"dinvb_{hp}_{th}"
                )
                nc.vector.tensor_copy(dinvb[:], dinv[:])

                def finalize():
                    # broadcast 1/D over partition bands via sel matmul,
                    # then yt = av * (1/D)
                    for c in range(2):
                        dbp = pj_pool.tile(
                            [128, 512], dt.float32, tag="pj", name=f"dbp_{hp}_{th}_{c}"
                        )
                        nc.tensor.matmul(
                            dbp[:], sel[c][:], dinvb[:], start=True, stop=True
                        )
                        dbs = dd_pool.tile(
                            [128, 512], dt.float32, tag="dbs", name=f"dbs_{hp}_{th}_{c}"
                        )
                        nc.vector.tensor_copy(dbs[:], dbp[:])
                        nc.vector.tensor_tensor(
                            yt_sb[hp][:, t0 + c * 512 : t0 + (c + 1) * 512],
                            av[c][:],
                            dbs[:],
                            op=Alu.mult,
                        )

                return finalize

            # ---------------- schedule ----------------
            for which in ("q", "k"):
                for cp in range(2):
                    qk_chunk(0, which, cp)
            for ub in range(4):
                v_chunk(ub)

            # v(j) must be emitted by filler slot j (consumed at iter j+1);
            # qk1 chunks slot in at 3/7/11/15 and finish within phase A.
            qk1 = [
                (lambda w=w, c=c: qk_chunk(1, w, c))
                for w in ("q", "k")
                for c in range(2)
            ]
            vs = [(lambda u=ub: v_chunk(u)) for ub in range(4, TB)]
            fillers_a = []
            for i in range(4):
                fillers_a.extend(vs[i * 3 : i * 3 + 3])
                fillers_a.append(qk1[i])
            fin_a = attention_phase(0, 0, fillers_a)

            fin_b = attention_phase(1, 0, [fin_a])

            fillers_c = [fin_b] + [(lambda t=tb: out_chunk(t)) for tb in range(0, 8)]
            fin_c = attention_phase(0, 1, fillers_c)

            fin_d = attention_phase(1, 1, [fin_c])
            fin_d()

            for tb in range(8, 16):
                out_chunk(tb)

    nc.finalize()
    return nc


def _prepare_in_maps(x, Wq, Wk, Wv, Wo):
    scale = 1.0 / math.sqrt(K)
    xT = [np.ascontiguousarray(x[b].T).astype(_BF16) for b in range(B)]
    in_maps = []
    for c in range(NCORES):
        b = c // 4
        g = c % 4
        sl = slice(g * 256, (g + 1) * 256)
        in_maps.append(
            {
                "xT": xT[b],
                "wq": np.ascontiguousarray((Wq[sl, :].astype(np.float64) * scale).T).astype(_BF16),
                "wk": np.ascontiguousarray(Wk[sl, :].T).astype(_BF16),
                "wv": np.ascontiguousarray(Wv[sl, :].T).astype(_BF16),
                "wo": np.ascontiguousarray(Wo[:, sl].T).astype(_BF16),
            }
        )
    return in_maps


def _gather(results, bo):
    out = np.zeros((B, T, K), dtype=np.float32)
    for b in range(B):
        acc = np.zeros((T, K), dtype=np.float32)
        for g in range(4):
            acc += results[b * 4 + g]["out"].astype(np.float32)
        out[b] = acc + bo.astype(np.float32)[None, :]
    return out


def _maybe_enable_ldw_opt():
    import os
    import concourse.bass_utils as bu

    if os.environ.get("LDWOPT", "0") != "1":
        return
    if getattr(bu, "_ldwopt_patched", False):
        return
    orig = bu.run_command

    def patched(argv, **kw):
        argv = [
            "--enable-ldw-opt=true" if a == "--enable-ldw-opt=false" else a
            for a in argv
        ]
        return orig(argv, **kw)

    bu.run_command = patched
    bu._ldwopt_patched = True


def run(x, Wq, Wk, Wv, Wo, bo, trace=False, tmpdir=None):
    from concourse.bass_utils import run_bass_kernel_spmd

    _maybe_enable_ldw_opt()
    _install_drain_split_patch()

    nc = build_program()
    in_maps = _prepare_in_maps(
        np.asarray(x), np.asarray(Wq), np.asarray(Wk), np.asarray(Wv), np.asarray(Wo)
    )
    res = run_bass_kernel_spmd(
        nc, in_maps, list(range(NCORES)), trace=trace, tmpdir=tmpdir
    )
    out = _gather(res.results, np.asarray(bo))
    return out, res


def kernel(x, Wq, Wk, Wv, Wo, bo):
    out, _ = run(x, Wq, Wk, Wv, Wo, bo, trace=False)
    return out


# revision 17
# speedup vs baseline: 1.6043x; 1.0278x over previous
"""Multi-head attention (16 heads, d_model=1024, T=2048, B=2) on 8 trn2 NeuronCores.

Sharding: core c -> batch c//4, head-group c%4 (4 heads of 64 dims each).
Each core computes q/k/v projections for its 4 heads on its batch, full
softmax attention for those heads, and a partial output projection
(row-parallel Wo).  Host sums the 4 partials per batch and adds the bias.

v2 design (vs baseline):
  - score matmuls (contract=64) emitted as adjacent h0/h64 row-tile pairs
    -> run concurrently on the two 64x128 PE tiles (~2x).
  - attn*V matmuls col-tiled: h0 -> psum partitions 0-63, h1 -> 64-127 of
    the same bank (~2x, and AV psum shrinks to 2 banks).
  - softmax denominators: fp16 running accumulation of the exp tiles on
    DVE/GpSimd, then a single ones-vector matmul per (head, chunk) reduces
    the 128 partitions; 1/D via ACT Ln + Exp(-x) (same act table set as
    the softmax Exp, so no table reloads).
  - V projection computed directly in [u, s] orientation (stationary xT
    block, moving Wv) - no PE transposes.
  - static software pipeline: projection / output chunks are interleaved
    into the ACT(exp)-bound attention phases as fillers; each phase's
    normalization is emitted as the first filler of the next phase so the
    PE never waits on the 1/D chain.
PSUM budget: st 2x[128,1024] (4 banks) + av 2x[128,512] (2) + pj 2 (2).
"""

import math
import os as _os

import numpy as np
import ml_dtypes

B = 2
T = 2048
K = 1024
H = 16
S = K // H  # 64
NCORES = 8
TB = T // 128  # 16 u-blocks
KT = K // 128  # 8 contraction tiles

_BF16 = ml_dtypes.bfloat16
_GPS_ACC = _os.environ.get("GPS_ACC", "0") == "1"


def _install_drain_split_patch():
    """walrus in this container rejects >1 sync-wait on the final tile drain;
    split the waits one-per-drain-instruction (all before the end barrier)."""
    import concourse.tile as tile
    import concourse.mybir as mybir
    from concourse.vector_clock import ScopedClock

    if getattr(tile.TileContext, "_drain_split_patched", False):
        return

    def _patched_dab(self, tick_clock, wait_clock):
        drain_inst = self.nc.sync.drain()
        wait_clock.add_sem_waits(
            drain_inst.ins, ScopedClock({None: tick_clock.global_clock})
        )
        si = drain_inst.ins.sync_info
        waits = list(si.on_wait) if si is not None else []
        if len(waits) > 1:
            si.on_wait = waits[:1]
            for w in waits[1:]:
                extra = self.nc.sync.drain()
                esi = extra.ins.sync_info
                if esi is None:
                    extra.ins.sync_info = mybir.SyncInfo(on_update=[], on_wait=[w])
                else:
                    esi.on_wait = [w]
        self.nc.all_engine_barrier()
        assert self.sems is not None
        popped = self.nc._tile_sem_poison_stack.pop()
        assert popped is self._sem_poison
        self.nc.clear_and_free_semaphores(list(self.sems.allocated().values()))
        self.nc.all_engine_barrier()

    tile.TileContext._drain_and_barrier = _patched_dab
    tile.TileContext._drain_split_patched = True


def build_program():
    import concourse.bass as bass
    import concourse.mybir as mybir
    import concourse.tile as tile
    from concourse import bacc

    dt = mybir.dt
    AF = mybir.ActivationFunctionType
    Alu = mybir.AluOpType

    nc = bacc.Bacc()

    xT = nc.dram_tensor("xT", [K, T], dt.bfloat16, kind="ExternalInput")
    wq = nc.dram_tensor("wq", [K, 256], dt.bfloat16, kind="ExternalInput")
    wk = nc.dram_tensor("wk", [K, 256], dt.bfloat16, kind="ExternalInput")
    wv = nc.dram_tensor("wv", [K, 256], dt.bfloat16, kind="ExternalInput")
    wo = nc.dram_tensor("wo", [256, K], dt.bfloat16, kind="ExternalInput")
    out = nc.dram_tensor("out", [T, K], dt.float16, kind="ExternalOutput")

    with tile.TileContext(nc) as tc:
        with (
            tc.tile_pool(name="xt", bufs=KT) as xt_pool,
            tc.tile_pool(name="w", bufs=3) as w_pool,
            tc.tile_pool(name="wo", bufs=2) as wo_pool,
            tc.tile_pool(name="qk", bufs=4) as qk_pool,
            tc.tile_pool(name="v", bufs=1) as v_pool,
            tc.tile_pool(name="es", bufs=6) as es_pool,
            tc.tile_pool(name="acc", bufs=4) as acc_pool,
            tc.tile_pool(name="dd", bufs=2) as dd_pool,
            tc.tile_pool(name="yt", bufs=2) as yt_pool,
            tc.tile_pool(name="osb", bufs=2) as osb_pool,
            tc.tile_pool(name="st", bufs=2, space="PSUM") as st_pool,
            tc.tile_pool(name="av", bufs=2, space="PSUM") as av_pool,
            tc.tile_pool(name="pj", bufs=2, space="PSUM") as pj_pool,
        ):
            # ---------------- loads ----------------
            w_sb = {}
            for name, dram in (("q", wq), ("k", wk), ("v", wv)):
                t = w_pool.tile([128, KT * 256], dt.bfloat16, tag="w", name=f"w_{name}")
                nc.sync.dma_start(
                    t[:].rearrange("p (a c) -> p a c", a=KT),
                    dram.rearrange("(a p) c -> p a c", p=128),
                )
                w_sb[name] = t

            xt = []
            for a in range(KT):
                t = xt_pool.tile([128, T], dt.bfloat16, tag="xt", name=f"xt_{a}")
                nc.sync.dma_start(t[:], xT[a * 128 : (a + 1) * 128, :])
                xt.append(t)

            wo_sb = []
            for i in range(2):
                t = wo_pool.tile([128, K], dt.bfloat16, tag="wo", name=f"wo_{i}")
                nc.sync.dma_start(t[:], wo[i * 128 : (i + 1) * 128, :])
                wo_sb.append(t)

            # selector matrices: broadcast 1/D rows (at partitions 0/32/64/96)
            # over the 64-partition bands of the head pair, per 512-chunk c.
            sel = []
            for c in range(2):
                s = v_pool.tile([128, 128], dt.bfloat16, tag=f"sel{c}", name=f"sel_{c}")
                nc.vector.memset(s[:], 0.0)
                nc.vector.memset(s[32 * c : 32 * c + 1, 0:64], 1.0)
                nc.vector.memset(s[64 + 32 * c : 64 + 32 * c + 1, 64:128], 1.0)
                sel.append(s)

            # denominator selectors: ones column at 0 (h0 -> out row 0) or
            # 64 (h1 -> out row 64), zero elsewhere -> (128,128)-mode reduce
            dsel = []
            for hl in range(2):
                s = v_pool.tile(
                    [128, 128], dt.float16, tag=f"dsel{hl}", name=f"dsel_{hl}"
                )
                nc.vector.memset(s[:], 0.0)
                nc.vector.memset(s[:, hl * 64 : hl * 64 + 1], 1.0)
                dsel.append(s)

            # V stationaries, zero-padded to M=128 per (ub, hp):
            # block layout [v_h0 (64) | zeros (64) | v_h1 (64)] of 192 cols so
            # h0 reads cols 0:128 = [v|0], h1 reads cols 64:192 = [0|v].
            VBLK = 192
            v_sb = v_pool.tile([128, TB * 2 * VBLK], dt.float16, tag="v", name="v_sb")
            nc.gpsimd.memset(v_sb[:], 0.0)

            qt_sb = [None, None]
            ktp_sb = {}  # (hp, hl) -> partition-padded K^T tile
            yt_sb = [
                yt_pool.tile([128, T], dt.bfloat16, tag="yt", name=f"yt_{hp}")
                for hp in range(2)
            ]

            # ---------------- projection chunks (fillers) ----------------
            def v_chunk(ub):
                """V[u-block ub, 256] = sum_a xt[a][:,ub].T @ wv[a], scattered
                into the zero-padded v_sb blocks (2 strided copies)."""
                pv = pj_pool.tile([128, 512], dt.float32, tag="pj", name=f"pv_{ub}")
                for a in range(KT):
                    nc.tensor.matmul(
                        pv[:, 0:256],
                        xt[a][:, ub * 128 : (ub + 1) * 128],
                        w_sb["v"][:, a * 256 : (a + 1) * 256],
                        start=(a == 0),
                        stop=(a == KT - 1),
                    )
                base = ub * 2 * VBLK
                win = v_sb[:, base : base + 2 * VBLK].rearrange(
                    "p (hp c) -> p hp c", hp=2
                )
                src = pv[:, 0:256].rearrange("p (hp h c) -> p hp h c", hp=2, h=2)
                # h0 of both head pairs -> block offsets {0, 192}+0
                nc.vector.tensor_copy(win[:, :, 0:64], src[:, :, 0, :])
                # h1 of both head pairs -> block offsets {0, 192}+128
                nc.vector.tensor_copy(win[:, :, 128:192], src[:, :, 1, :])

            def qk_chunk(hp, which, cp):
                """q or k projection for head pair hp, column pair cp
                (2 x 512 t-columns); stationary weight slice shared across
                the c pair, accumulating a=0..7 into two pj banks.
                K evacuates into two partition-padded tiles (other head's
                64 partitions zeroed) so score matmuls run at full 128
                contract in uniform (128,128) mode."""
                if which == "q" and qt_sb[hp] is None:
                    qt_sb[hp] = qk_pool.tile(
                        [128, T], dt.bfloat16, tag="qk", name=f"qt_{hp}"
                    )
                if which == "k" and (hp, 0) not in ktp_sb:
                    for hl in range(2):
                        t = qk_pool.tile(
                            [128, T], dt.bfloat16, tag=f"ktp{hl}", name=f"ktp_{hp}_{hl}"
                        )
                        nc.gpsimd.memset(t[(1 - hl) * 64 : (2 - hl) * 64, :], 0.0)
                        ktp_sb[(hp, hl)] = t
                ps = [
                    pj_pool.tile(
                        [128, 512], dt.float32, tag="pj", name=f"p{which}_{hp}_{cp}_{i}"
                    )
                    for i in range(2)
                ]
                for a in range(KT):
                    for i in range(2):
                        nc.tensor.matmul(
                            ps[i][:],
                            w_sb[which][:, a * 256 + hp * 128 : a * 256 + hp * 128 + 128],
                            xt[a][:, cp * 1024 + i * 512 : cp * 1024 + (i + 1) * 512],
                            start=(a == 0),
                            stop=(a == KT - 1),
                        )
                for i in range(2):
                    tsl = slice(cp * 1024 + i * 512, cp * 1024 + (i + 1) * 512)
                    if which == "q":
                        nc.vector.tensor_copy(qt_sb[hp][:, tsl], ps[i][:])
                    else:
                        for hl in range(2):
                            psl = slice(hl * 64, (hl + 1) * 64)
                            nc.vector.tensor_copy(
                                ktp_sb[(hp, hl)][psl, tsl], ps[i][psl, :]
                            )

            def out_chunk(tb):
                """output projection for t-block tb; DVE evac; DMA out."""
                osb = osb_pool.tile([128, K], dt.float16, tag="osb", name=f"osb_{tb}")
                pso = [
                    pj_pool.tile([128, 512], dt.float32, tag="pj", name=f"po_{tb}_{i}")
                    for i in range(2)
                ]
                for hp in range(2):
                    for oc in range(2):
                        nc.tensor.matmul(
                            pso[oc][:],
                            yt_sb[hp][:, tb * 128 : (tb + 1) * 128],
                            wo_sb[hp][:, oc * 512 : (oc + 1) * 512],
                            start=(hp == 0),
                            stop=(hp == 1),
                        )
                for oc in range(2):
                    nc.vector.tensor_copy(osb[:, oc * 512 : (oc + 1) * 512], pso[oc][:])
                nc.sync.dma_start(out[tb * 128 : (tb + 1) * 128, :], osb[:])

            # ---------------- attention phase ----------------
            def attention_phase(hp, th, fillers):
                """One (head pair, t-half) block.  fillers: callables emitting
                one chunk each, pumped one per ub iteration.  Returns a
                finalize closure (normalization) to pump into the NEXT phase
                (it must be emitted before that phase's first AV matmul)."""
                t0 = th * 1024
                qt = qt_sb[hp]
                av = [
                    av_pool.tile(
                        [128, 512], dt.float32, tag="av", name=f"av_{hp}_{th}_{c}"
                    )
                    for c in range(2)
                ]
                acc = {}

                def scores(ub):
                    sts = [
                        st_pool.tile(
                            [128, 1024], dt.float32, tag="st",
                            name=f"st_{hp}_{th}_{ub}_{hl}",
                        )
                        for hl in range(2)
                    ]
                    # full-contract (zero-padded) -> uniform (128,128) mode
                    for c in range(2):
                        for hl in range(2):
                            nc.tensor.matmul(
                                sts[hl][:, c * 512 : (c + 1) * 512],
                                ktp_sb[(hp, hl)][:, ub * 128 : (ub + 1) * 128],
                                qt[:, t0 + c * 512 : t0 + (c + 1) * 512],
                                start=True,
                                stop=True,
                            )
                    return sts

                def expify(ub, sts):
                    ess = []
                    for hl in range(2):
                        e = es_pool.tile(
                            [128, 1024], dt.float16, tag="es",
                            name=f"e_{hp}_{th}_{ub}_{hl}",
                        )
                        nc.scalar.activation(e[:], sts[hl][:], AF.Exp)
                        ess.append(e)
                    return ess

                def avmm(ub, ess):
                    # M=128 zero-padded stationaries; h0 and h1 accumulate
                    # additively into the same bank (pad rows contribute 0)
                    for c in range(2):
                        for hl in range(2):
                            vcol = ub * 2 * VBLK + hp * VBLK + hl * 64
                            nc.tensor.matmul(
                                av[c][:],
                                v_sb[:, vcol : vcol + 128],
                                ess[hl][:, c * 512 : (c + 1) * 512],
                                start=(ub == 0 and hl == 0),
                                stop=(ub == TB - 1 and hl == 1),
                                skip_group_check=True,
                            )

                def accum(ub, ess):
                    for hl in range(2):
                        eng = nc.gpsimd if (_GPS_ACC and hl == 1) else nc.vector
                        if ub == 0:
                            a0 = acc_pool.tile(
                                [128, 1024], dt.float16, tag="acc",
                                name=f"acc_{hp}_{th}_{hl}_0",
                            )
                            eng.tensor_copy(a0[:], ess[hl][:])
                            acc[hl] = a0
                        else:
                            nxt = acc_pool.tile(
                                [128, 1024], dt.float16, tag="acc",
                                name=f"acc_{hp}_{th}_{hl}_{ub}",
                            )
                            eng.tensor_tensor(
                                nxt[:], acc[hl][:], ess[hl][:], op=Alu.add
                            )
                            acc[hl] = nxt

                # software pipeline: AV lags scores by one ub
                fi = 0
                prev = None
                for ub in range(TB):
                    sts = scores(ub)
                    ess = expify(ub, sts)
                    if prev is not None:
                        avmm(prev[0], prev[1])
                        accum(prev[0], prev[1])
                    if fi < len(fillers):
                        fillers[fi]()
                        fi += 1
                    prev = (ub, ess)
                avmm(prev[0], prev[1])
                accum(prev[0], prev[1])
                while fi < len(fillers):
                    fillers[fi]()
                    fi += 1

                # denominators: padded-selector matmul partition reduction
                # (D_h0 -> out row 0, D_h1 -> row 64, same bank, same mode)
                pjd = [
                    pj_pool.tile(
                        [128, 512], dt.float32, tag="pj", name=f"pjd_{hp}_{th}_{c}"
                    )
                    for c in range(2)
                ]
                for c in range(2):
                    for hl in range(2):
                        nc.tensor.matmul(
                            pjd[c][:],
                            dsel[hl][:],
                            acc[hl][:, c * 512 : (c + 1) * 512],
                            start=(hl == 0),
                            stop=(hl == 1),
                            skip_group_check=True,
                        )
                drows = dd_pool.tile(
                    [128, 512], dt.float32, tag="drows", name=f"drows_{hp}_{th}"
                )
                nc.gpsimd.memset(drows[:], 1.0)
                for c in range(2):
                    for hl in range(2):
                        p = hl * 64 + c * 32
                        nc.vector.tensor_copy(
                            drows[p : p + 1, :], pjd[c][hl * 64 : hl * 64 + 1, :]
                        )
                # 1/D via single-op approx reciprocal (~18 bits, plenty)
                dinv = dd_pool.tile(
                    [128, 512], dt.float32, tag="dln", name=f"dinv_{hp}_{th}"
                )
                nc.vector.reciprocal_approx_fast(dinv[:], drows[:])
                dinvb = dd_pool.tile(
                    [128, 512], dt.bfloat16, tag="dinvb", name=f"dinvb_{hp}_{th}"
                )
                nc.vector.tensor_copy(dinvb[:], dinv[:])

                def finalize():
                    # broadcast 1/D over partition bands via sel matmul,
                    # then yt = av * (1/D)
                    for c in range(2):
                        dbp = pj_pool.tile(
                            [128, 512], dt.float32, tag="pj", name=f"dbp_{hp}_{th}_{c}"
                        )
                        nc.tensor.matmul(
                            dbp[:], sel[c][:], dinvb[:], start=True, stop=True
                        )
                        dbs = dd_pool.tile(
                            [128, 512], dt.float32, tag="dbs", name=f"dbs_{hp}_{th}_{c}"
                        )
                        nc.vector.tensor_copy(dbs[:], dbp[:])
                        nc.vector.tensor_tensor(
                            yt_sb[hp][:, t0 + c * 512 : t0 + (c + 1) * 512],
                            av[c][:],
                            dbs[:],
                            op=Alu.mult,
                        )

                return finalize

            # ---------------- schedule ----------------
            # phase A (hp0, th0) only needs qt0/kt0 t-columns 0-1023 (cp0);
            # everything else is deferred into the phases as fillers, with
            # deadlines: v(j) by filler slot j (consumed at iter j+1),
            # k0cp1 by A-slot 7 (scores ub>=8), qk1 by end of A, k1cp1 by
            # B-slot 7, q0cp1 by end of B (phase C), q1cp1 by end of C.
            qk_chunk(0, "q", 0)
            qk_chunk(0, "k", 0)
            for ub in range(4):
                v_chunk(ub)

            qkc = lambda hp, w, c: (lambda: qk_chunk(hp, w, c))
            vs = [(lambda u=ub: v_chunk(u)) for ub in range(4, TB)]
            fillers_a = (
                vs[0:3]
                + [qkc(0, "k", 1)]
                + vs[3:6]
                + [qkc(1, "q", 0)]
                + vs[6:9]
                + [qkc(1, "k", 0)]
                + vs[9:12]
                + [qkc(0, "q", 1)]
            )
            fin_a = attention_phase(0, 0, fillers_a)

            fin_b = attention_phase(1, 0, [fin_a, qkc(1, "k", 1), qkc(1, "q", 1)])

            fillers_c = [fin_b] + [(lambda t=tb: out_chunk(t)) for tb in range(0, 8)]
            fin_c = attention_phase(0, 1, fillers_c)

            fin_d = attention_phase(1, 1, [fin_c])
            fin_d()

            for tb in range(8, 16):
                out_chunk(tb)

    nc.finalize()
    return nc


def _prepare_in_maps(x, Wq, Wk, Wv, Wo):
    scale = 1.0 / math.sqrt(K)
    xT = [np.ascontiguousarray(x[b].T).astype(_BF16) for b in range(B)]
    in_maps = []
    for c in range(NCORES):
        b = c // 4
        g = c % 4
        sl = slice(g * 256, (g + 1) * 256)
        in_maps.append(
            {
                "xT": xT[b],
                "wq": np.ascontiguousarray((Wq[sl, :].astype(np.float64) * scale).T).astype(_BF16),
                "wk": np.ascontiguousarray(Wk[sl, :].T).astype(_BF16),
                "wv": np.ascontiguousarray(Wv[sl, :].T).astype(_BF16),
                "wo": np.ascontiguousarray(Wo[:, sl].T).astype(_BF16),
            }
        )
    return in_maps


def _gather(results, bo):
    out = np.zeros((B, T, K), dtype=np.float32)
    for b in range(B):
        acc = np.zeros((T, K), dtype=np.float32)
        for g in range(4):
            acc += results[b * 4 + g]["out"].astype(np.float32)
        out[b] = acc + bo.astype(np.float32)[None, :]
    return out


def _maybe_enable_ldw_opt():
    import os
    import concourse.bass_utils as bu

    if os.environ.get("LDWOPT", "0") != "1":
        return
    if getattr(bu, "_ldwopt_patched", False):
        return
    orig = bu.run_command

    def patched(argv, **kw):
        argv = [
            "--enable-ldw-opt=true" if a == "--enable-ldw-opt=false" else a
            for a in argv
        ]
        return orig(argv, **kw)

    bu.run_command = patched
    bu._ldwopt_patched = True


def run(x, Wq, Wk, Wv, Wo, bo, trace=False, tmpdir=None):
    from concourse.bass_utils import run_bass_kernel_spmd

    _maybe_enable_ldw_opt()
    _install_drain_split_patch()

    nc = build_program()
    in_maps = _prepare_in_maps(
        np.asarray(x), np.asarray(Wq), np.asarray(Wk), np.asarray(Wv), np.asarray(Wo)
    )
    res = run_bass_kernel_spmd(
        nc, in_maps, list(range(NCORES)), trace=trace, tmpdir=tmpdir
    )
    out = _gather(res.results, np.asarray(bo))
    return out, res


def kernel(x, Wq, Wk, Wv, Wo, bo):
    out, _ = run(x, Wq, Wk, Wv, Wo, bo, trace=False)
    return out
